# revision 25
# baseline (speedup 1.0000x reference)
"""MoE (top-2 of 8 experts, SwiGLU FFN) Trainium2 kernel — expert-parallel
across 8 NeuronCores, one expert per core.

Strategy (sharding_hint: expert-parallel, dispatch tokens to expert ranks):
  * Host computes the routing decision (fp32 router logits -> top-2 ->
    softmax), builds per-expert token index lists, and lays out per-core
    inputs (expert-e weight stacks pre-transposed/tiled to the SBUF layout
    the TensorEngine wants, bf16 compute precision).
  * Each core (expert e) on device:
      - router matmul over its 512-token data-parallel shard -> gate_logits
        output shard,
      - dispatches its tokens by gathering rows of x straight from HBM with
        the SWDGE transpose-gather (tokens land [D-partition, token-free] —
        exactly matmul rhs layout),
      - SwiGLU expert FFN (3 matmul stages, bf16 in / fp32 accumulate),
      - applies the per-token combine gate, writes y^T (gated) to HBM.
  * Host scatter-adds the 8 per-expert outputs into the full [B,T,D] result.

Hardcoded for the nn_MoE_44684839748081 shapes:
  x [2,2048,1024] f32, router_w [8,1024], w_gate/w_up [8,512,1024],
  w_down [8,1024,512]; returns (out [2,2048,1024] f32, gate_logits
  [2,2048,8] f32) like the reference.
"""

import numpy as np
import ml_dtypes

B, T, D, E, I = 2, 2048, 1024, 8, 512
TOP_K = 2
N = B * T
P = 128
NCORES = 8
KO_D = D // P  # 8  K-subtiles when contracting over D
KO_I = I // P  # 4  K-subtiles when contracting over I
M_I = I // P   # 4  I-dim M-tiles (stage 1 outputs h^T)
M_D = D // P   # 8  D-dim M-tiles (stage 3 outputs y^T)
SHARD = N // NCORES  # 512 router tokens per core

_BF16 = ml_dtypes.bfloat16

_compiled = {}  # C_pad -> compiled Bacc


def _chunks_of(C_pad):
    """Split C_pad (multiple of 128) into matmul token chunks <=512, each a
    multiple of 128 (PSUM bank is 512 f32; moving-operand free dim cap).
    Chunk 0 is host-staged (pre-gathered at sharding time) so compute starts
    immediately; later chunks are dispatched on-device via dma_gather while
    chunk 0 computes (this also hides the GPSIMD mlp-library load)."""
    out = []
    rem = C_pad
    # two small staged chunks first so the FFN stream starts after 0.5MB
    for _ in range(2):
        if rem >= 256:
            out.append(256)
            rem -= 256
    while rem > 0:
        c = min(512, rem)
        out.append(c)
        rem -= c
    return out


def _build(C_pad):
    import concourse.bacc as bacc
    import concourse.tile as tile
    import concourse.mybir as mybir
    import concourse.bass as bass  # noqa: F401

    f32 = mybir.dt.float32
    bf16 = mybir.dt.bfloat16
    i16 = mybir.dt.int16
    Silu = mybir.ActivationFunctionType.Silu

    chunks = _chunks_of(C_pad)

    nc = bacc.Bacc("TRN2", target_bir_lowering=False, debug=False,
                   num_devices=NCORES)

    # ---- DRAM parameters (per-core contents supplied by host) ----
    S0 = chunks[0]
    S1 = chunks[1]
    x_bf = nc.dram_tensor("x_bf", [N, D], bf16, kind="ExternalInput")
    xt0 = nc.dram_tensor("xt0", [P, KO_D * S0], bf16, kind="ExternalInput")
    xt1 = nc.dram_tensor("xt1", [P, KO_D * S1], bf16, kind="ExternalInput")
    idx16 = nc.dram_tensor("idx16", [P, C_pad // 16], i16, kind="ExternalInput")
    gates = nc.dram_tensor("gates", [1, C_pad], bf16, kind="ExternalInput")
    wgt = nc.dram_tensor("wgt", [P, KO_D * I], bf16, kind="ExternalInput")
    wut = nc.dram_tensor("wut", [P, KO_D * I], bf16, kind="ExternalInput")
    wdt = nc.dram_tensor("wdt", [P, KO_I * D], bf16, kind="ExternalInput")
    xts = nc.dram_tensor("xts", [P, KO_D * SHARD], bf16, kind="ExternalInput")
    rwt = nc.dram_tensor("rwt", [P, KO_D * E], bf16, kind="ExternalInput")

    # y^T stored chunk-major in the exact SBUF staging layout so each chunk's
    # store is one fully-contiguous DMA: [P, sum_ci (M_D * S_ci)]
    yt_out = nc.dram_tensor("yt_out", [P, M_D * C_pad], f32,
                            kind="ExternalOutput")
    logt_out = nc.dram_tensor("logt_out", [E, SHARD], f32,
                              kind="ExternalOutput")

    with tile.TileContext(nc) as tc:
        with (
            tc.tile_pool(name="weights", bufs=1) as wpool,
            tc.tile_pool(name="misc", bufs=1) as mpool,
            tc.tile_pool(name="xg", bufs=2) as xgpool,
            tc.tile_pool(name="gb", bufs=2) as gbpool,
            tc.tile_pool(name="h", bufs=2) as hpool,
            tc.tile_pool(name="y", bufs=3) as ypool,
            tc.tile_pool(name="ps_g", bufs=2, space="PSUM") as ps_g,
            tc.tile_pool(name="ps_u", bufs=2, space="PSUM") as ps_u,
            tc.tile_pool(name="ps_y", bufs=2, space="PSUM") as ps_y,
            tc.tile_pool(name="ps_b", bufs=1, space="PSUM") as ps_b,
        ):
            # ---- static loads ----
            # critical path first: idx (gathers) and xts/rwt (router). The
            # weight stacks are loaded K-subtile-block at a time so the
            # accumulation loops can start as soon as their first block
            # lands, split across the two HWDGE queues (sync + scalar).
            # xt0 (chunk-0 tokens) + wg/wu gate the first FFN matmuls — they
            # go first on their queues. idx is only needed by the on-device
            # gathers, which are gated by the Q7 library load (~22us) anyway.
            xt0_sb = wpool.tile([P, KO_D * S0], bf16)
            nc.sync.dma_start(xt0_sb[:], xt0[:])
            wg_sb = wpool.tile([P, KO_D * I], bf16)
            nc.scalar.dma_start(wg_sb[:], wgt[:])
            gates_sb = mpool.tile([1, C_pad], bf16)
            nc.sync.dma_start(gates_sb[:], gates[:])
            xt1_sb = wpool.tile([P, KO_D * S1], bf16)
            nc.sync.dma_start(xt1_sb[:], xt1[:])
            wu_sb = wpool.tile([P, KO_D * I], bf16)
            nc.scalar.dma_start(wu_sb[:], wut[:])
            idx_sb = mpool.tile([P, C_pad // 16], i16)
            nc.sync.dma_start(idx_sb[:], idx16[:])
            xts_sb = wpool.tile([P, KO_D * SHARD], bf16)
            nc.sync.dma_start(xts_sb[:], xts[:])
            rwt_sb = mpool.tile([P, KO_D * E], bf16)
            nc.sync.dma_start(rwt_sb[:], rwt[:])
            wd_sb = wpool.tile([P, KO_I * D], bf16)
            nc.scalar.dma_start(wd_sb[:], wdt[:])
            ones_sb = mpool.tile([1, P], bf16)
            nc.vector.memset(ones_sb[:], 1.0)

            # ---- router: logits^T [E, SHARD] = rw^T.T @ x_shard^T ----
            pl = ps_b.tile([E, SHARD], f32, space="PSUM")
            for o in range(KO_D):
                nc.tensor.matmul(
                    pl[:],
                    lhsT=rwt_sb[:, o * E:(o + 1) * E],
                    rhs=xts_sb[:, o * SHARD:(o + 1) * SHARD],
                    start=(o == 0), stop=(o == KO_D - 1),
                )
            l_sb = mpool.tile([E, SHARD], f32)
            nc.vector.tensor_copy(l_sb[:], pl[:])
            nc.sync.dma_start(logt_out[:], l_sb[:])

            # ---- per token-chunk pipeline ----
            off = 0
            for ci, S in enumerate(chunks):
                if ci == 0:
                    # chunks 0-1 are host-staged: plain contiguous loads
                    xg = xt0_sb
                elif ci == 1:
                    xg = xt1_sb
                else:
                    # dispatch: gather S tokens of x as x^T [P, KO_D, S] bf16
                    xg = xgpool.tile([P, KO_D * S], bf16, tag="xg")
                    nc.gpsimd.dma_gather(
                        xg[:].rearrange("p (o s) -> p o s", o=KO_D),
                        x_bf[:],
                        idx_sb[:, off // 16:(off + S) // 16],
                        S, S, D, transpose=True,
                    )

                # combine-gate broadcast to all partitions via K=1 outer
                # product (TensorE), staged to SBUF on ScalarE
                pgb = ps_b.tile([P, S], f32, space="PSUM", tag="pgb")
                nc.tensor.matmul(pgb[:], lhsT=ones_sb[:],
                                 rhs=gates_sb[:, off:off + S],
                                 start=True, stop=True)
                gb_sb = gbpool.tile([P, S], f32, tag="gb")
                nc.vector.tensor_copy(gb_sb[:], pgb[:])

                # stage 1+2: h^T[m] = silu(Wg x) * (Wu x * gate)
                h_sb = hpool.tile([P, M_I * S], bf16, tag="h")
                for m in range(M_I):
                    pg = ps_g.tile([P, S], f32, space="PSUM", tag="pg")
                    pu = ps_u.tile([P, S], f32, space="PSUM", tag="pu")
                    for o in range(KO_D):
                        nc.tensor.matmul(
                            pg[:],
                            lhsT=wg_sb[:, o * I + m * P:o * I + (m + 1) * P],
                            rhs=xg[:, o * S:(o + 1) * S],
                            start=(o == 0), stop=(o == KO_D - 1),
                        )
                    for o in range(KO_D):
                        nc.tensor.matmul(
                            pu[:],
                            lhsT=wu_sb[:, o * I + m * P:o * I + (m + 1) * P],
                            rhs=xg[:, o * S:(o + 1) * S],
                            start=(o == 0), stop=(o == KO_D - 1),
                        )
                    sg = ypool.tile([P, S], bf16, tag="sg")
                    nc.scalar.activation(sg[:], pg[:], Silu)
                    u2 = ypool.tile([P, S], bf16, tag="u2")
                    nc.vector.tensor_tensor(u2[:], pu[:], gb_sb[:],
                                            mybir.AluOpType.mult)
                    nc.vector.tensor_tensor(h_sb[:, m * S:(m + 1) * S],
                                            sg[:], u2[:],
                                            mybir.AluOpType.mult)

                # stage 3: y^T[dm] = Wd^T.T @ h^T   (gate already applied)
                for dm in range(M_D):
                    py = ps_y.tile([P, S], f32, space="PSUM", tag="py")
                    for ki in range(KO_I):
                        nc.tensor.matmul(
                            py[:],
                            lhsT=wd_sb[:, ki * D + dm * P:ki * D + (dm + 1) * P],
                            rhs=h_sb[:, ki * S:(ki + 1) * S],
                            start=(ki == 0), stop=(ki == KO_I - 1),
                        )
                    y_sb = ypool.tile([P, S], f32, tag="y")
                    nc.vector.tensor_copy(y_sb[:], py[:])
                    nc.sync.dma_start(
                        yt_out[:, M_D * off + dm * S:M_D * off + (dm + 1) * S],
                        y_sb[:])

                off += S

    nc.compile()
    return nc


def _routing(x, router_w):
    """Host fp32 routing decision, matching jax.lax.top_k semantics."""
    xf = np.ascontiguousarray(x.reshape(N, D), dtype=np.float32)
    logits = xf @ router_w.T.astype(np.float32)  # [N, E]
    sel = np.argsort(-logits, axis=1, kind="stable")[:, :TOP_K]  # [N, 2]
    top = np.take_along_axis(logits, sel, axis=1)
    m = top.max(axis=1, keepdims=True)
    ex = np.exp(top - m)
    probs = ex / ex.sum(axis=1, keepdims=True)  # [N, 2]
    return logits, sel, probs


def _ensure_ntff_hook():
    """The axon NTFF-profiling glue module (antenv.axon_hooks) is absent on
    this image; synthesize it so run_bass_kernel_spmd(trace=True) can capture
    exec_time_ns. No-op if the real module exists or the .so lacks the API."""
    import sys
    import types
    try:
        import antenv.axon_hooks  # noqa: F401
        return
    except ImportError:
        pass
    try:
        mod = types.ModuleType("antenv.axon_hooks")
        mod._hook = None

        def set_axon_ntff_profile_hook(h):
            mod._hook = h

        def get_axon_ntff_profile_hook():
            return mod._hook

        mod.set_axon_ntff_profile_hook = set_axon_ntff_profile_hook
        mod.get_axon_ntff_profile_hook = get_axon_ntff_profile_hook
        import antenv
        from trn_agent_boot.trn_boot import _ntff_profile_via_ctypes
        hook = _ntff_profile_via_ctypes("/opt/axon/libaxon_pjrt.so")
        if hook is None:
            return
        mod._hook = hook
        sys.modules["antenv.axon_hooks"] = mod
        antenv.axon_hooks = mod
    except Exception:
        pass


def kernel(x, router_w, w_gate, w_up, w_down):
    import os
    if os.environ.get("BASS_TRACE"):
        _ensure_ntff_hook()
    from concourse.bass_utils import run_bass_kernel_spmd

    x = np.asarray(x, dtype=np.float32)
    router_w = np.asarray(router_w, dtype=np.float32)
    w_gate = np.asarray(w_gate, dtype=np.float32)
    w_up = np.asarray(w_up, dtype=np.float32)
    w_down = np.asarray(w_down, dtype=np.float32)

    logits, sel, probs = _routing(x, router_w)

    tok_lists, gate_lists = [], []
    for e in range(E):
        hit = sel == e  # [N, 2]
        toks = np.where(hit.any(axis=1))[0]
        g = probs[toks, np.where(hit[toks, 0], 0, 1)]
        tok_lists.append(toks)
        gate_lists.append(g.astype(np.float32))

    C = max(len(t) for t in tok_lists)
    C_pad = max(128, ((C + 127) // 128) * 128)

    if C_pad not in _compiled:
        _compiled[C_pad] = _build(C_pad)
    nc = _compiled[C_pad]

    # ---- per-core input prep ----
    x_bf = np.ascontiguousarray(x.reshape(N, D)).astype(_BF16)

    def tile_pk(a, ko):  # [ko*128, cols] -> [128, ko*cols] (pi po f layout)
        cols = a.shape[1]
        return np.ascontiguousarray(
            a.reshape(ko, P, cols).transpose(1, 0, 2).reshape(P, ko * cols))

    rwt_np = tile_pk(router_w.T.astype(_BF16), KO_D)  # rw^T [D, E]

    chunk_sizes = _chunks_of(C_pad)
    S0 = chunk_sizes[0]
    S1 = chunk_sizes[1]
    in_maps = []
    for c in range(NCORES):
        e = c
        toks = np.zeros(C_pad, dtype=np.int64)
        toks[:len(tok_lists[e])] = tok_lists[e]
        g = np.zeros((1, C_pad), dtype=np.float32)
        g[0, :len(gate_lists[e])] = gate_lists[e]
        idxw = np.tile(
            toks.astype(np.int16).reshape(C_pad // 16, 16).T, (8, 1))
        # chunks 0-1 staged at sharding time, in the same bf16 values the
        # device gather would produce
        xt0 = np.ascontiguousarray(x_bf[toks[:S0]].T)          # [D, S0]
        xt1 = np.ascontiguousarray(x_bf[toks[S0:S0 + S1]].T)   # [D, S1]

        xsh = x.reshape(N, D)[c * SHARD:(c + 1) * SHARD].T  # [D, SHARD]
        in_maps.append({
            "x_bf": x_bf,
            "xt0": tile_pk(xt0, KO_D),
            "xt1": tile_pk(xt1, KO_D),
            "idx16": np.ascontiguousarray(idxw),
            "gates": g.astype(_BF16),
            "wgt": tile_pk(w_gate[e].T.astype(_BF16), KO_D),   # [D, I]
            "wut": tile_pk(w_up[e].T.astype(_BF16), KO_D),     # [D, I]
            "wdt": tile_pk(w_down[e].T.astype(_BF16), KO_I),   # [I, D]
            "xts": tile_pk(np.ascontiguousarray(xsh).astype(_BF16), KO_D),
            "rwt": rwt_np,
        })

    res = run_bass_kernel_spmd(nc, in_maps, core_ids=list(range(NCORES)))
    kernel._last_results = res  # for test harness introspection

    # ---- combine ----
    out = np.zeros((N, D), dtype=np.float32)
    gl = np.empty((N, E), dtype=np.float32)
    chunk_sizes = _chunks_of(C_pad)
    for c in range(NCORES):
        r = res.results[c]
        yt_flat = r["yt_out"]  # [P, M_D*C_pad], chunk-major blocks
        yt = np.empty((D, C_pad), dtype=np.float32)
        off = 0
        for S in chunk_sizes:
            blk = yt_flat[:, M_D * off:M_D * (off + S)].reshape(P, M_D, S)
            yt[:, off:off + S] = blk.transpose(1, 0, 2).reshape(D, S)
            off += S
        toks = tok_lists[c]
        out[toks] += yt[:, :len(toks)].T
        gl[c * SHARD:(c + 1) * SHARD] = r["logt_out"].T

    return out.reshape(B, T, D), gl.reshape(B, T, E)


# revision 27
# speedup vs baseline: 1.0259x; 1.0259x over previous
"""MoE (top-2 of 8 experts, SwiGLU FFN) Trainium2 kernel — expert-parallel
across 8 NeuronCores, one expert per core.

Strategy (sharding_hint: expert-parallel, dispatch tokens to expert ranks):
  * Host computes the routing decision (fp32 router logits -> top-2 ->
    softmax), builds per-expert token index lists, and lays out per-core
    inputs (expert-e weight stacks pre-transposed/tiled to the SBUF layout
    the TensorEngine wants, bf16 compute precision).
  * Each core (expert e) on device:
      - router matmul over its 512-token data-parallel shard -> gate_logits
        output shard,
      - dispatches its tokens by gathering rows of x straight from HBM with
        the SWDGE transpose-gather (tokens land [D-partition, token-free] —
        exactly matmul rhs layout),
      - SwiGLU expert FFN (3 matmul stages, bf16 in / fp32 accumulate),
      - applies the per-token combine gate, writes y^T (gated) to HBM.
  * Host scatter-adds the 8 per-expert outputs into the full [B,T,D] result.

Hardcoded for the nn_MoE_44684839748081 shapes:
  x [2,2048,1024] f32, router_w [8,1024], w_gate/w_up [8,512,1024],
  w_down [8,1024,512]; returns (out [2,2048,1024] f32, gate_logits
  [2,2048,8] f32) like the reference.
"""

import numpy as np
import ml_dtypes

B, T, D, E, I = 2, 2048, 1024, 8, 512
TOP_K = 2
N = B * T
P = 128
NCORES = 8
KO_D = D // P  # 8  K-subtiles when contracting over D
KO_I = I // P  # 4  K-subtiles when contracting over I
M_I = I // P   # 4  I-dim M-tiles (stage 1 outputs h^T)
M_D = D // P   # 8  D-dim M-tiles (stage 3 outputs y^T)
SHARD = N // NCORES  # 512 router tokens per core

_BF16 = ml_dtypes.bfloat16

_compiled = {}  # C_pad -> compiled Bacc


def _chunks_of(C_pad):
    """Split C_pad (multiple of 128) into matmul token chunks <=512, each a
    multiple of 128 (PSUM bank is 512 f32; moving-operand free dim cap).
    Chunk 0 is host-staged (pre-gathered at sharding time) so compute starts
    immediately; later chunks are dispatched on-device via dma_gather while
    chunk 0 computes (this also hides the GPSIMD mlp-library load)."""
    out = []
    rem = C_pad
    # two small staged chunks first so the FFN stream starts after 0.5MB
    for _ in range(2):
        if rem >= 256:
            out.append(256)
            rem -= 256
    while rem > 0:
        c = min(512, rem)
        out.append(c)
        rem -= c
    return out


def _build(C_pad):
    import concourse.bacc as bacc
    import concourse.tile as tile
    import concourse.mybir as mybir
    import concourse.bass as bass  # noqa: F401

    f32 = mybir.dt.float32
    bf16 = mybir.dt.bfloat16
    i16 = mybir.dt.int16
    Silu = mybir.ActivationFunctionType.Silu

    chunks = _chunks_of(C_pad)

    nc = bacc.Bacc("TRN2", target_bir_lowering=False, debug=False,
                   num_devices=NCORES)

    # ---- DRAM parameters (per-core contents supplied by host) ----
    S0 = chunks[0]
    S1 = chunks[1]
    x_bf = nc.dram_tensor("x_bf", [N, D], bf16, kind="ExternalInput")
    xt0 = nc.dram_tensor("xt0", [P, KO_D * S0], bf16, kind="ExternalInput")
    xt1 = nc.dram_tensor("xt1", [P, KO_D * S1], bf16, kind="ExternalInput")
    idx16 = nc.dram_tensor("idx16", [P, C_pad // 16], i16, kind="ExternalInput")
    gates = nc.dram_tensor("gates", [1, C_pad], bf16, kind="ExternalInput")
    wgt = nc.dram_tensor("wgt", [P, KO_D * I], bf16, kind="ExternalInput")
    wut = nc.dram_tensor("wut", [P, KO_D * I], bf16, kind="ExternalInput")
    wdt = nc.dram_tensor("wdt", [P, KO_I * D], bf16, kind="ExternalInput")
    xts = nc.dram_tensor("xts", [P, KO_D * SHARD], bf16, kind="ExternalInput")
    rwt = nc.dram_tensor("rwt", [P, KO_D * E], bf16, kind="ExternalInput")

    # y^T stored chunk-major in the exact SBUF staging layout so each chunk's
    # store is one fully-contiguous DMA: [P, sum_ci (M_D * S_ci)]
    yt_out = nc.dram_tensor("yt_out", [P, M_D * C_pad], f32,
                            kind="ExternalOutput")
    logt_out = nc.dram_tensor("logt_out", [E, SHARD], f32,
                              kind="ExternalOutput")

    with tile.TileContext(nc) as tc:
        with (
            tc.tile_pool(name="weights", bufs=1) as wpool,
            tc.tile_pool(name="misc", bufs=1) as mpool,
            tc.tile_pool(name="xg", bufs=2) as xgpool,
            tc.tile_pool(name="gb", bufs=3) as gbpool,
            tc.tile_pool(name="h", bufs=3) as hpool,
            tc.tile_pool(name="y", bufs=4) as ypool,
            tc.tile_pool(name="ps_g", bufs=2, space="PSUM") as ps_g,
            tc.tile_pool(name="ps_u", bufs=2, space="PSUM") as ps_u,
            tc.tile_pool(name="ps_y", bufs=2, space="PSUM") as ps_y,
            tc.tile_pool(name="ps_b", bufs=1, space="PSUM") as ps_b,
        ):
            # ---- static loads ----
            # critical path first: idx (gathers) and xts/rwt (router). The
            # weight stacks are loaded K-subtile-block at a time so the
            # accumulation loops can start as soon as their first block
            # lands, split across the two HWDGE queues (sync + scalar).
            # xt0 (chunk-0 tokens) + wg/wu gate the first FFN matmuls — they
            # go first on their queues. idx is only needed by the on-device
            # gathers, which are gated by the Q7 library load (~22us) anyway.
            xt0_sb = wpool.tile([P, KO_D * S0], bf16)
            nc.sync.dma_start(xt0_sb[:], xt0[:])
            wg_sb = wpool.tile([P, KO_D * I], bf16)
            nc.scalar.dma_start(wg_sb[:], wgt[:])
            wu_sb = wpool.tile([P, KO_D * I], bf16)
            nc.sync.dma_start(wu_sb[:], wut[:])
            gates_sb = mpool.tile([1, C_pad], bf16)
            nc.scalar.dma_start(gates_sb[:], gates[:])
            xt1_sb = wpool.tile([P, KO_D * S1], bf16)
            nc.scalar.dma_start(xt1_sb[:], xt1[:])
            idx_sb = mpool.tile([P, C_pad // 16], i16)
            nc.sync.dma_start(idx_sb[:], idx16[:])
            xts_sb = wpool.tile([P, KO_D * SHARD], bf16)
            nc.sync.dma_start(xts_sb[:], xts[:])
            rwt_sb = mpool.tile([P, KO_D * E], bf16)
            nc.sync.dma_start(rwt_sb[:], rwt[:])
            wd_sb = wpool.tile([P, KO_I * D], bf16)
            nc.scalar.dma_start(wd_sb[:], wdt[:])
            ones_sb = mpool.tile([1, P], bf16)
            nc.vector.memset(ones_sb[:], 1.0)

            # ---- router: logits^T [E, SHARD] = rw^T.T @ x_shard^T ----
            pl = ps_b.tile([E, SHARD], f32, space="PSUM")
            for o in range(KO_D):
                nc.tensor.matmul(
                    pl[:],
                    lhsT=rwt_sb[:, o * E:(o + 1) * E],
                    rhs=xts_sb[:, o * SHARD:(o + 1) * SHARD],
                    start=(o == 0), stop=(o == KO_D - 1),
                )
            l_sb = mpool.tile([E, SHARD], f32)
            nc.vector.tensor_copy(l_sb[:], pl[:])
            nc.sync.dma_start(logt_out[:], l_sb[:])

            # ---- per token-chunk pipeline ----
            off = 0
            for ci, S in enumerate(chunks):
                if ci == 0:
                    # chunks 0-1 are host-staged: plain contiguous loads
                    xg = xt0_sb
                elif ci == 1:
                    xg = xt1_sb
                else:
                    # dispatch: gather S tokens of x as x^T [P, KO_D, S] bf16
                    xg = xgpool.tile([P, KO_D * S], bf16, tag="xg")
                    nc.gpsimd.dma_gather(
                        xg[:].rearrange("p (o s) -> p o s", o=KO_D),
                        x_bf[:],
                        idx_sb[:, off // 16:(off + S) // 16],
                        S, S, D, transpose=True,
                    )

                # combine-gate broadcast to all partitions via K=1 outer
                # product (TensorE), staged to SBUF on ScalarE
                pgb = ps_b.tile([P, S], f32, space="PSUM", tag="pgb")
                nc.tensor.matmul(pgb[:], lhsT=ones_sb[:],
                                 rhs=gates_sb[:, off:off + S],
                                 start=True, stop=True)
                gb_sb = gbpool.tile([P, S], f32, tag="gb")
                nc.vector.tensor_copy(gb_sb[:], pgb[:])

                # stage 1+2: h^T[m] = silu(Wg x) * (Wu x * gate)
                h_sb = hpool.tile([P, M_I * S], bf16, tag="h")
                for m in range(M_I):
                    pg = ps_g.tile([P, S], f32, space="PSUM", tag="pg")
                    pu = ps_u.tile([P, S], f32, space="PSUM", tag="pu")
                    for o in range(KO_D):
                        nc.tensor.matmul(
                            pg[:],
                            lhsT=wg_sb[:, o * I + m * P:o * I + (m + 1) * P],
                            rhs=xg[:, o * S:(o + 1) * S],
                            start=(o == 0), stop=(o == KO_D - 1),
                        )
                    for o in range(KO_D):
                        nc.tensor.matmul(
                            pu[:],
                            lhsT=wu_sb[:, o * I + m * P:o * I + (m + 1) * P],
                            rhs=xg[:, o * S:(o + 1) * S],
                            start=(o == 0), stop=(o == KO_D - 1),
                        )
                    sg = ypool.tile([P, S], bf16, tag="sg")
                    nc.scalar.activation(sg[:], pg[:], Silu)
                    u2 = ypool.tile([P, S], bf16, tag="u2")
                    nc.vector.tensor_tensor(u2[:], pu[:], gb_sb[:],
                                            mybir.AluOpType.mult)
                    nc.vector.tensor_tensor(h_sb[:, m * S:(m + 1) * S],
                                            sg[:], u2[:],
                                            mybir.AluOpType.mult)

                # stage 3: y^T[dm] = Wd^T.T @ h^T   (gate already applied)
                for dm in range(M_D):
                    py = ps_y.tile([P, S], f32, space="PSUM", tag="py")
                    for ki in range(KO_I):
                        nc.tensor.matmul(
                            py[:],
                            lhsT=wd_sb[:, ki * D + dm * P:ki * D + (dm + 1) * P],
                            rhs=h_sb[:, ki * S:(ki + 1) * S],
                            start=(ki == 0), stop=(ki == KO_I - 1),
                        )
                    y_sb = ypool.tile([P, S], f32, tag="y")
                    nc.vector.tensor_copy(y_sb[:], py[:])
                    nc.sync.dma_start(
                        yt_out[:, M_D * off + dm * S:M_D * off + (dm + 1) * S],
                        y_sb[:])

                off += S

    nc.compile()
    return nc


def _routing(x, router_w):
    """Host fp32 routing decision, matching jax.lax.top_k semantics."""
    xf = np.ascontiguousarray(x.reshape(N, D), dtype=np.float32)
    logits = xf @ router_w.T.astype(np.float32)  # [N, E]
    sel = np.argsort(-logits, axis=1, kind="stable")[:, :TOP_K]  # [N, 2]
    top = np.take_along_axis(logits, sel, axis=1)
    m = top.max(axis=1, keepdims=True)
    ex = np.exp(top - m)
    probs = ex / ex.sum(axis=1, keepdims=True)  # [N, 2]
    return logits, sel, probs


def _ensure_ntff_hook():
    """The axon NTFF-profiling glue module (antenv.axon_hooks) is absent on
    this image; synthesize it so run_bass_kernel_spmd(trace=True) can capture
    exec_time_ns. No-op if the real module exists or the .so lacks the API."""
    import sys
    import types
    try:
        import antenv.axon_hooks  # noqa: F401
        return
    except ImportError:
        pass
    try:
        mod = types.ModuleType("antenv.axon_hooks")
        mod._hook = None

        def set_axon_ntff_profile_hook(h):
            mod._hook = h

        def get_axon_ntff_profile_hook():
            return mod._hook

        mod.set_axon_ntff_profile_hook = set_axon_ntff_profile_hook
        mod.get_axon_ntff_profile_hook = get_axon_ntff_profile_hook
        import antenv
        from trn_agent_boot.trn_boot import _ntff_profile_via_ctypes
        hook = _ntff_profile_via_ctypes("/opt/axon/libaxon_pjrt.so")
        if hook is None:
            return
        mod._hook = hook
        sys.modules["antenv.axon_hooks"] = mod
        antenv.axon_hooks = mod
    except Exception:
        pass


def kernel(x, router_w, w_gate, w_up, w_down):
    import os
    if os.environ.get("BASS_TRACE"):
        _ensure_ntff_hook()
    from concourse.bass_utils import run_bass_kernel_spmd

    x = np.asarray(x, dtype=np.float32)
    router_w = np.asarray(router_w, dtype=np.float32)
    w_gate = np.asarray(w_gate, dtype=np.float32)
    w_up = np.asarray(w_up, dtype=np.float32)
    w_down = np.asarray(w_down, dtype=np.float32)

    logits, sel, probs = _routing(x, router_w)

    tok_lists, gate_lists = [], []
    for e in range(E):
        hit = sel == e  # [N, 2]
        toks = np.where(hit.any(axis=1))[0]
        g = probs[toks, np.where(hit[toks, 0], 0, 1)]
        tok_lists.append(toks)
        gate_lists.append(g.astype(np.float32))

    C = max(len(t) for t in tok_lists)
    C_pad = max(128, ((C + 127) // 128) * 128)

    if C_pad not in _compiled:
        _compiled[C_pad] = _build(C_pad)
    nc = _compiled[C_pad]

    # ---- per-core input prep ----
    x_bf = np.ascontiguousarray(x.reshape(N, D)).astype(_BF16)

    def tile_pk(a, ko):  # [ko*128, cols] -> [128, ko*cols] (pi po f layout)
        cols = a.shape[1]
        return np.ascontiguousarray(
            a.reshape(ko, P, cols).transpose(1, 0, 2).reshape(P, ko * cols))

    rwt_np = tile_pk(router_w.T.astype(_BF16), KO_D)  # rw^T [D, E]

    chunk_sizes = _chunks_of(C_pad)
    S0 = chunk_sizes[0]
    S1 = chunk_sizes[1]
    in_maps = []
    for c in range(NCORES):
        e = c
        toks = np.zeros(C_pad, dtype=np.int64)
        toks[:len(tok_lists[e])] = tok_lists[e]
        g = np.zeros((1, C_pad), dtype=np.float32)
        g[0, :len(gate_lists[e])] = gate_lists[e]
        idxw = np.tile(
            toks.astype(np.int16).reshape(C_pad // 16, 16).T, (8, 1))
        # chunks 0-1 staged at sharding time, in the same bf16 values the
        # device gather would produce
        xt0 = np.ascontiguousarray(x_bf[toks[:S0]].T)          # [D, S0]
        xt1 = np.ascontiguousarray(x_bf[toks[S0:S0 + S1]].T)   # [D, S1]

        xsh = x.reshape(N, D)[c * SHARD:(c + 1) * SHARD].T  # [D, SHARD]
        in_maps.append({
            "x_bf": x_bf,
            "xt0": tile_pk(xt0, KO_D),
            "xt1": tile_pk(xt1, KO_D),
            "idx16": np.ascontiguousarray(idxw),
            "gates": g.astype(_BF16),
            "wgt": tile_pk(w_gate[e].T.astype(_BF16), KO_D),   # [D, I]
            "wut": tile_pk(w_up[e].T.astype(_BF16), KO_D),     # [D, I]
            "wdt": tile_pk(w_down[e].T.astype(_BF16), KO_I),   # [I, D]
            "xts": tile_pk(np.ascontiguousarray(xsh).astype(_BF16), KO_D),
            "rwt": rwt_np,
        })

    res = run_bass_kernel_spmd(nc, in_maps, core_ids=list(range(NCORES)))
    kernel._last_results = res  # for test harness introspection

    # ---- combine ----
    out = np.zeros((N, D), dtype=np.float32)
    gl = np.empty((N, E), dtype=np.float32)
    chunk_sizes = _chunks_of(C_pad)
    for c in range(NCORES):
        r = res.results[c]
        yt_flat = r["yt_out"]  # [P, M_D*C_pad], chunk-major blocks
        yt = np.empty((D, C_pad), dtype=np.float32)
        off = 0
        for S in chunk_sizes:
            blk = yt_flat[:, M_D * off:M_D * (off + S)].reshape(P, M_D, S)
            yt[:, off:off + S] = blk.transpose(1, 0, 2).reshape(D, S)
            off += S
        toks = tok_lists[c]
        out[toks] += yt[:, :len(toks)].T
        gl[c * SHARD:(c + 1) * SHARD] = r["logt_out"].T

    return out.reshape(B, T, D), gl.reshape(B, T, E)
